# revision 19
# baseline (speedup 1.0000x reference)
"""Trainium2 Bass kernel for AgreementRouting (dynamic routing / capsule-style).

Full-input contract: kernel(u_predict[64,2048,32,16] f32, b[2048,32] f32) -> v[64,32,16] f32.
Internally shards batch (64) across 8 NeuronCores (8 batch elems per core).

Per-core design (B_loc=8, L=2048, H=32, D=16, HD=512), fp16 operands with fp32
PSUM accumulation, two groups of 4 batch elems:
  load: one gpsimd casting DMA per b: u fp32 HBM -> nat[b] [128l, (16t, 512hd)] fp16
  uT:   DMA crossbar transposes (sync queue) -> 64 chunk tiles [128hd, 128l] per b
        (no PE transposes, no DVE evacuation copies)
  b_batch lives in PSUM: one bank per b, preloaded with b via an identity
        matmul, agreement matmuls accumulate start=False across all 3 iters
  agreement: upd[l,h] += sum_hd uT[hd,l]*V[hd,h] -- 64 MMs (N=32) per b/iter,
        weights=uT chunks, rhs=block-diag V slices
  softmax: exp (Scalar, PSUM src) -> reduce/recip (DVE) -> mult -> c fp16
  ws (flipped): weights=c[:,t,:] [128l,32h'], rhs=nat[b][:,t,:] streamed with
        (d,h2)-permuted columns, N=512 -> PSUM [64=(2b,32h'),(16d,32h2)] per
        b-pair; 16 MMs per b/pass
  extraction (per pair): mask-mult + contiguous inner reduce -> s32 [64,16];
        squash on [64,1]; V rebuilt via PE transpose + broadcast matmul + one
        masked mult; final v DMA'd per b from s32 layout
"""

import sys
import os

sys.path.insert(0, "/opt/trn_rl_repo")

import numpy as np
from contextlib import ExitStack

B, L, H, D = 64, 2048, 32, 16
NCORES = 8
BLOC = B // NCORES  # 8
HD = H * D  # 512
NT = L // 128  # 16 l-chunks
NKC = HD // 128  # 4 hd chunks
NITER = 3
EPS = 1e-8
GRP = 4

_NC_CACHE = {}
LAST_EXEC_NS = None
LAST_RESULTS = None
LAST_TRACE_DIR = None
_TRACE = False


def _consts():
    p = np.arange(128)
    # ind_mask[p, NKC*? -> (k, h)] = 1 iff h == 8k + p//16
    ind_mask = np.zeros((128, NKC * H), np.float16)
    for k in range(NKC):
        ind_mask[p, H * k + 8 * k + p // 16] = 1.0
    # h16[p, g] = 1 iff p//16 == g ; it16 = h16.T
    h16 = (p[:, None] // 16 == np.arange(8)[None, :]).astype(np.float32)
    it16 = np.ascontiguousarray(h16.T)
    id16 = np.eye(128, dtype=np.float16)
    return {
        "ind_mask": ind_mask,
        "h16": h16,
        "it16": it16,
        "id16": id16,
    }


def _emit(ctx, tc, t_in, t_out):
    import concourse.mybir as mybir

    nc = tc.nc
    f32 = mybir.dt.float32
    f16 = mybir.dt.float16
    AF = mybir.ActivationFunctionType
    ALU = mybir.AluOpType
    AX = mybir.AxisListType

    u_ap = t_in["u"]
    b_ap = t_in["b"]
    im_ap = t_in["ind_mask"]
    h16_ap = t_in["h16"]
    it16_ap = t_in["it16"]
    id16_ap = t_in["id16"]
    vout_ap = t_out["v_out"]

    cpool = ctx.enter_context(tc.tile_pool(name="cpool", bufs=1))
    p_nat = ctx.enter_context(tc.tile_pool(name="p_nat", bufs=7))
    p_uT = ctx.enter_context(tc.tile_pool(name="p_uT", bufs=18))
    p_e = ctx.enter_context(tc.tile_pool(name="p_e", bufs=3))
    p_c = ctx.enter_context(tc.tile_pool(name="p_c", bufs=5))
    p_z = ctx.enter_context(tc.tile_pool(name="p_z", bufs=6))
    p_prod = ctx.enter_context(tc.tile_pool(name="p_prod", bufs=2))
    p_s = ctx.enter_context(tc.tile_pool(name="p_s", bufs=4))
    p_sq = ctx.enter_context(tc.tile_pool(name="p_sq", bufs=16))
    p_v = ctx.enter_context(tc.tile_pool(name="p_v", bufs=4))
    p_vm = ctx.enter_context(tc.tile_pool(name="p_vm", bufs=4))
    ps_bb = ctx.enter_context(tc.tile_pool(name="ps_bb", bufs=GRP, space="PSUM"))
    ps_ws = ctx.enter_context(tc.tile_pool(name="ps_ws", bufs=1, space="PSUM"))
    ps_vs = ctx.enter_context(tc.tile_pool(name="ps_vs", bufs=1, space="PSUM"))
    ps_tr = ctx.enter_context(tc.tile_pool(name="ps_tr", bufs=2, space="PSUM"))

    # ---- constants
    im_t = cpool.tile([128, NKC * H], f16, name="im_t")
    nc.sync.dma_start(im_t[:], im_ap)
    h16_t = cpool.tile([128, 8], f32, name="h16_t")
    nc.sync.dma_start(h16_t[:], h16_ap)
    it16_t = cpool.tile([8, 128], f32, name="it16_t")
    nc.sync.dma_start(it16_t[:], it16_ap)
    id16_t = cpool.tile([128, 128], f16, name="id16_t")
    nc.sync.dma_start(id16_t[:], id16_ap)
    bin_t = cpool.tile([128, NT * H], f32, name="bin_t")
    nc.sync.dma_start(
        bin_t[:].rearrange("p (t h) -> p t h", t=NT),
        b_ap.rearrange("(t p) h -> p t h", p=128),
    )
    bin16 = cpool.tile([128, NT * H], f16, name="bin16")
    nc.vector.tensor_copy(bin16[:], bin_t[:])

    # ---- c0 = softmax(b) over h (shared; logits bounded, no max-subtraction)
    e0 = p_e.tile([128, NT * H], f32, name="e0", tag="e")
    nc.scalar.activation(e0[:], bin_t[:], AF.Exp)
    z0 = p_z.tile([128, NT], f32, name="z0", tag="z")
    nc.vector.reduce_sum(z0[:], e0[:].rearrange("p (t h) -> p t h", t=NT), AX.X)
    r0 = p_z.tile([128, NT], f32, name="r0", tag="z")
    nc.vector.reciprocal(r0[:], z0[:])
    c0 = cpool.tile([128, NT * H], f16, name="c0")
    nc.vector.tensor_tensor(
        c0[:].rearrange("p (t h) -> p t h", t=NT),
        e0[:].rearrange("p (t h) -> p t h", t=NT),
        r0[:].unsqueeze(2).broadcast_to((128, NT, H)),
        ALU.mult,
    )

    st = {}  # per-b state
    pst = {}  # per-pair state

    def emit_load(b):
        nat = p_nat.tile([128, NT * HD], f16, name="nat", tag="nat")
        nat3 = nat[:].rearrange("p (t hd) -> p t hd", t=NT)
        src = u_ap[b].rearrange("(t p) h d -> p t (h d)", p=128)
        nh = NT // 2
        nc.gpsimd.dma_start(nat3[:, :nh, :], src[:, :nh, :])
        nc.gpsimd.dma_start(nat3[:, nh:, :], src[:, nh:, :])
        st[b] = {"nat": nat}

    def emit_trans(b):
        u32 = mybir.dt.uint32
        nat3 = st[b]["nat"][:].rearrange("p (t hd) -> p t hd", t=NT)
        uT = []
        for k in range(NKC):
            ut = p_uT.tile([128, L], f16, name="ut", tag="uT")
            uT.append(ut)
        for k in range(NKC):
            for tq in range(NT // 4):
                ptr = ps_tr.tile([128, 4 * 128], f16, name="ptr", tag="ptr", padded_shape=[128, 1024])
                for j in range(4):
                    nc.tensor.transpose(
                        ptr[:, 128 * j : 128 * (j + 1)],
                        nat3[:, 4 * tq + j, 128 * k : 128 * (k + 1)],
                        id16_t[:],
                    )
                dst = uT[k][:, 512 * tq : 512 * (tq + 1)]
                nc.vector.tensor_copy(dst.bitcast(u32), ptr[:].bitcast(u32))
        st[b]["uT"] = uT

    def emit_preload(b):
        bank = ps_bb.tile([128, NT * H], f32, name="bank", tag="bb")
        nc.tensor.matmul(bank[:], id16_t[:], bin16[:], start=True, stop=False)
        st[b]["bank"] = bank

    def emit_ws(b, c_ap, wsp):
        """u-stationary: O[hd, (b2,k,h')] += sum_l u[l,hd] c[l,h']; weights are
        nat l-chunk slices, rhs = c[:, t, :]; k-sequential accumulation groups
        in the pair's quarter of the shared bank."""
        q = b % GRP
        nat3 = st[b]["nat"][:].rearrange("p (t hd) -> p t hd", t=NT)
        for k in range(NKC):
            col = (q * NKC + k) * H
            for t in range(NT):
                nc.tensor.matmul(
                    wsp[:, col : col + H],
                    nat3[:, t, 128 * k : 128 * (k + 1)],
                    c_ap[:, t, :],
                    start=(t == 0),
                    stop=(t == NT - 1),
                )

    def emit_extract(pr, bs, last):
        """per-pair, hd layout: masked mult + reduce -> s_sb [128,(2b,4k)];
        squash via tiny h16/it16 matmuls; V rebuilt with one broadcast mult."""
        wsp = pst[pr]["wsp"]
        b2o = (bs[0] % 4) // 2  # which half of the shared bank
        half = wsp[:, b2o * 2 * NKC * H : (b2o + 1) * 2 * NKC * H]
        prod = p_prod.tile([128, 2 * NKC * H], f32, name="prod", tag="prod")
        nc.vector.tensor_tensor(
            prod[:].rearrange("p (b k h) -> p b k h", b=2, k=NKC),
            half.rearrange("p (b k h) -> p b k h", b=2, k=NKC),
            im_t[:].rearrange("p (k h) -> p k h", k=NKC)
            .unsqueeze(1)
            .broadcast_to((128, 2, NKC, H)),
            ALU.mult,
        )
        s_sb = p_s.tile([128, 2 * NKC], f32, name="s_sb", tag="s")
        nc.vector.reduce_sum(
            s_sb[:], prod[:].rearrange("p (c h) -> p c h", h=H), AX.X
        )
        s2 = p_s.tile([128, 2 * NKC], f32, name="s2", tag="s")
        nc.vector.tensor_tensor(s2[:], s_sb[:], s_sb[:], ALU.mult)
        # sqT[g, (b2, k)] = ||s_{b, h=8k+g}||^2
        sqT = ps_vs.tile([8, 2 * NKC], f32, name="sqT", tag="vs", padded_shape=[128, 512])
        nc.tensor.matmul(sqT[:], h16_t[:], s2[:], start=True, stop=True)
        t1 = p_sq.tile([8, 2 * NKC], f32, name="t1", tag="sq")
        nc.vector.tensor_scalar_add(t1[:], sqT[:], 1.0)
        teps = p_sq.tile([8, 2 * NKC], f32, name="teps", tag="sq")
        nc.vector.tensor_scalar_add(teps[:], sqT[:], EPS)

        rt = p_sq.tile([8, 2 * NKC], f32, name="rt", tag="sq")
        nc.scalar.activation(rt[:], teps[:], AF.Sqrt)
        den = p_sq.tile([8, 2 * NKC], f32, name="den", tag="sq")
        nc.vector.tensor_tensor(den[:], t1[:], rt[:], ALU.mult)
        rd = p_sq.tile([8, 2 * NKC], f32, name="rd", tag="sq")
        nc.vector.reciprocal(rd[:], den[:])
        fT = p_sq.tile([8, 2 * NKC], f32, name="fT", tag="sq")
        nc.vector.tensor_tensor(fT[:], sqT[:], rd[:], ALU.mult)
        fexp = ps_vs.tile([128, 2 * NKC], f32, name="fexp", tag="vs", padded_shape=[128, 512])
        nc.tensor.matmul(fexp[:], it16_t[:], fT[:], start=True, stop=True)
        vT = p_v.tile([128, 2 * NKC], f32, name="vT", tag="v")
        nc.vector.tensor_tensor(vT[:], s_sb[:], fexp[:], ALU.mult)
        if last:
            for j, b in enumerate(bs):
                nc.gpsimd.dma_start(
                    vout_ap[b]
                    .rearrange("h d -> (h d)")
                    .rearrange("(c p) -> p c", p=128),
                    vT[:, NKC * j : NKC * (j + 1)],
                )
            pst[pr]["vms"] = None
            return
        vms = p_vm.tile([128, 2 * NKC * H], f16, name="vms", tag="vm")
        nc.vector.tensor_tensor(
            vms[:].rearrange("p (b k h) -> p b k h", b=2, k=NKC),
            vT[:].rearrange("p (b k) -> p b k", b=2)
            .unsqueeze(3)
            .broadcast_to((128, 2, NKC, H)),
            im_t[:].rearrange("p (k h) -> p k h", k=NKC)
            .unsqueeze(1)
            .broadcast_to((128, 2, NKC, H)),
            ALU.mult,
        )
        pst[pr]["vms"] = vms

    def emit_agree(b, it):
        uT = st[b]["uT"]
        bank3 = st[b]["bank"][:].rearrange("p (t h) -> p t h", t=NT)
        vms4 = pst[(b // 2) % 2]["vms"][:].rearrange(
            "p (b k h) -> p b k h", b=2, k=NKC
        )
        b2 = b % 2
        for t in range(NT):
            for k in range(NKC):
                nc.tensor.matmul(
                    bank3[:, t, :],
                    uT[k][:, 128 * t : 128 * (t + 1)],
                    vms4[:, b2, k, :],
                    start=False,
                    stop=(it == NITER - 1 and k == NKC - 1),
                )

    def emit_softmax(b):
        bank = st[b]["bank"]
        e = p_e.tile([128, NT * H], f32, name="e", tag="e")
        nc.scalar.activation(e[:], bank[:], AF.Exp)
        z = p_z.tile([128, NT], f32, name="z", tag="z")
        nc.vector.reduce_sum(z[:], e[:].rearrange("p (t h) -> p t h", t=NT), AX.X)
        r = p_z.tile([128, NT], f32, name="r", tag="z")
        nc.vector.reciprocal(r[:], z[:])
        c_t = p_c.tile([128, NT * H], f16, name="ct", tag="c")
        nc.vector.tensor_tensor(
            c_t[:].rearrange("p (t h) -> p t h", t=NT),
            e[:].rearrange("p (t h) -> p t h", t=NT),
            r[:].unsqueeze(2).broadcast_to((128, NT, H)),
            ALU.mult,
        )
        st[b]["c"] = c_t

    c0_3 = c0[:].rearrange("p (t h) -> p t h", t=NT)

    # loads b0..b6 upfront (7 nat bufs); b7 emitted late so its pool-buffer
    # wait does not block later gpsimd work in-queue
    for b in range(BLOC - 1):
        emit_load(b)

    for g in range(BLOC // GRP):
        bs = list(range(g * GRP, (g + 1) * GRP))
        for b in bs:
            emit_preload(b)
        # init ws pass with shared c0, transposes interleaved per b
        wsp = ps_ws.tile([128, GRP * NKC * H], f32, name="wsp", tag="ws", padded_shape=[128, 512])
        for pr in range(2):
            pair = bs[2 * pr : 2 * pr + 2]
            pst[pr] = {"wsp": wsp}
            for b in pair:
                emit_ws(b, c0_3, wsp)
                emit_trans(b)
            if pr == 0:
                warm0 = p_sq.tile([8, 2 * NKC], f32, name="warm0", tag="sq")
                nc.scalar.activation(warm0[:], h16_t[:8, :8], AF.Sqrt)
            emit_extract(pr, pair, False)
        warm3 = p_sq.tile([8, 2 * NKC], f32, name="warm3", tag="sq")
        nc.scalar.activation(warm3[:], h16_t[:8, :8], AF.Exp)
        for it in range(NITER):
            last = it == NITER - 1
            emit_agree(bs[0], it)
            emit_agree(bs[1], it)
            emit_softmax(bs[0])
            emit_agree(bs[2], it)
            emit_softmax(bs[1])
            emit_agree(bs[3], it)
            wspi = ps_ws.tile([128, GRP * NKC * H], f32, name="wsp", tag="ws", padded_shape=[128, 512])
            pst[0] = {"wsp": wspi}
            pst[1] = {"wsp": wspi}
            emit_ws(bs[0], st[bs[0]]["c"][:].rearrange("p (t h) -> p t h", t=NT), wspi)
            emit_softmax(bs[2])
            emit_ws(bs[1], st[bs[1]]["c"][:].rearrange("p (t h) -> p t h", t=NT), wspi)
            emit_softmax(bs[3])
            # prewarm the Sqrt table while the ws matmuls stream
            warm = p_sq.tile([8, 2 * NKC], f32, name="warm", tag="sq")
            nc.scalar.activation(warm[:], h16_t[:8, :8], AF.Sqrt)
            emit_ws(bs[2], st[bs[2]]["c"][:].rearrange("p (t h) -> p t h", t=NT), wspi)
            emit_ws(bs[3], st[bs[3]]["c"][:].rearrange("p (t h) -> p t h", t=NT), wspi)
            if g == 0 and it == NITER - 1:
                emit_load(7)
            emit_extract(0, bs[0:2], last)
            emit_extract(1, bs[2:4], last)
            # prewarm Exp for the next softmax block
            warm2 = p_sq.tile([8, 2 * NKC], f32, name="warm2", tag="sq")
            nc.scalar.activation(warm2[:], h16_t[:8, :8], AF.Exp)


def _get_nc():
    if "nc" in _NC_CACHE:
        return _NC_CACHE["nc"]
    from concourse import bacc
    import concourse.tile as tile
    import concourse.mybir as mybir

    f32 = mybir.dt.float32
    f16 = mybir.dt.float16
    nc = bacc.Bacc("TRN2", target_bir_lowering=False, debug=False)
    t_in = {}
    in_shapes = {
        "u": ([BLOC, L, H, D], f32),
        "b": ([L, H], f32),
        "ind_mask": ([128, NKC * H], f16),
        "h16": ([128, 8], f32),
        "it16": ([8, 128], f32),
        "id16": ([128, 128], f16),
    }
    for name, (shape, dt_) in in_shapes.items():
        t_in[name] = nc.dram_tensor(name, shape, dt_, kind="ExternalInput").ap()
    vout = nc.dram_tensor("v_out", [BLOC, H, D], f32, kind="ExternalOutput").ap()

    with tile.TileContext(nc) as tc:
        with ExitStack() as ctx:
            _emit(ctx, tc, t_in, {"v_out": vout})
    nc.compile()
    _NC_CACHE["nc"] = nc
    return nc


def kernel(u_predict, b):
    global LAST_EXEC_NS, LAST_RESULTS
    u = np.ascontiguousarray(np.asarray(u_predict, dtype=np.float32))
    bq = np.ascontiguousarray(np.asarray(b, dtype=np.float32))
    assert u.shape == (B, L, H, D), u.shape
    assert bq.shape == (L, H), bq.shape

    nc = _get_nc()
    consts = _consts()
    in_maps = []
    for i in range(NCORES):
        m = {"u": np.ascontiguousarray(u[i * BLOC : (i + 1) * BLOC]), "b": bq}
        m.update(consts)
        in_maps.append(m)

    from concourse.bass_utils import run_bass_kernel_spmd

    global LAST_TRACE_DIR
    kw = {}
    if _TRACE:
        import tempfile

        LAST_TRACE_DIR = tempfile.mkdtemp(prefix="bass_trace_")
        kw["tmpdir"] = LAST_TRACE_DIR
    res = run_bass_kernel_spmd(nc, in_maps, list(range(NCORES)), trace=_TRACE, **kw)
    LAST_EXEC_NS = res.exec_time_ns
    LAST_RESULTS = res
    out = np.concatenate([r["v_out"] for r in res.results], axis=0)
    return out.astype(np.float32)
